# revision 30
# baseline (speedup 1.0000x reference)
"""Trainium2 Bass kernel: batched chamfer-style metric (nn_Metric_56985626083917).

Reference computation per batch b (B=8, N=M=4096, D=3):
    sqd[n,m] = |pred_n - gt_m|^2   (clamped >= 0)
    dist1 = sqrt(min_m sqd)  [N] ; dist2 = sqrt(min_n sqd)  [M]
    loss_b = mean(dist1)+mean(dist2) + 3*(mean(top2048(dist1))+mean(top2048(dist2)))
    out = mean_b loss_b

Strategy: data-parallel, one batch per NeuronCore (8 cores).
Per core the device computes zt[n,m] = -sqd[n,m] via K=13 fp16 matmuls with
error-compensated hi/lo splits (fp32-grade accuracy at full fp16 PE rate):
    zt = sum_c 2*p_c*g_c - |p|^2 - |g|^2
slots: per coord c 3 products (ph*gh, ph*gl, pl*gh; pl*gl ~ 2^-22 dropped),
plus 2 slots for -|p|^2 (hi/lo vs ones) and 2 slots for -|g|^2 -> K=13.

Main loop: statically unrolled over 32 pred tiles x 2 PSUM halves of 2048
gt columns (double-buffered). No hardware loop: tc.For_i executes at
~5-8us per instruction on this stack (measured; likely SW-decode), so
full unrolling is ~7x faster despite the larger NEFF. Benchmark reps are
additional unrolled copies of the body (V0 scheme):
    PE  : 4 matmuls per half -> PSUM [128, 2048] fp32 (zt half-tile)
    ACT : copy PSUM halves -> merged SBUF fp16 tile C [128, 4096]
    DVE : rowmax (dist1) as a TREE — 2x-mode fp16 tensor_tensor max of
          the two halves (1.1us) then a 2048-wide tensor_reduce (2.1us),
          instead of a 4096-wide 1x reduce (4.4us); plus one 2x-mode
          fp16 tensor_tensor max into run2 (dist2)
(A DVE/Pool hybrid that moved half the colmax folds to gpsimd
partition_all_reduce measured ~25% SLOWER despite a favorable per-op
microbenchmark — interleaved gpsimd reads contend for SBUF ports.)

Device outputs raw row maxes [128, 32] fp32 and the column maxes folded
on-device to [1, 4096] fp16 — keeping transfers tiny; the host (O(N)
work) does relu/sqrt, means, and exact top-k via np.partition, then
averages the 8 losses.
"""

import os
import sys

import numpy as np

for _p in ("/opt/trn_rl_repo",):
    if os.path.isdir(_p) and _p not in sys.path:
        sys.path.insert(0, _p)

import concourse.bass as bass  # noqa: E402
import concourse.bass_isa as bass_isa  # noqa: E402
import concourse.mybir as mybir  # noqa: E402
import concourse.tile as tile  # noqa: E402
from concourse import bacc  # noqa: E402
from concourse.bass_utils import run_bass_kernel_spmd  # noqa: E402

B = 8
N = 4096  # pred points per batch
M = 4096  # gt points per batch
P = 128  # partitions
KSLOTS = 13
NTILE = N // P  # 32
MCHUNK = 2048  # gt columns per PSUM half
NCHUNK = M // MCHUNK  # 2
MM_N = 512  # moving free dim per matmul (<= 1 PSUM bank)
K1 = N // 2  # top-k count (PERCENT=0.5)
WEIGHT = 3.0

F16 = mybir.dt.float16
F32 = mybir.dt.float32
Alu = mybir.AluOpType
X = mybir.AxisListType.X

LAST_RESULT = None
_CACHE = {}


def _build_nc(reps=1):
    nc = bacc.Bacc(
        "TRN2", target_bir_lowering=False, debug=False, num_devices=B
    )
    a_in = nc.dram_tensor("A", [KSLOTS, N], F16, kind="ExternalInput")
    g_in = nc.dram_tensor("G", [KSLOTS, M], F16, kind="ExternalInput")
    r1_out = nc.dram_tensor("R1OUT", [P, NTILE], F32, kind="ExternalOutput")
    c2_out = nc.dram_tensor("C2OUT", [1, M], F16, kind="ExternalOutput")

    with tile.TileContext(nc) as tc:
        for _ in range(reps):
            _body(nc, tc, a_in, g_in, r1_out, c2_out)
    nc.compile()
    return nc


def _body(nc, tc, a_in, g_in, r1_out, c2_out):
    from contextlib import ExitStack

    with ExitStack() as ctx:
        io = ctx.enter_context(tc.tile_pool(name="io", bufs=1))
        cpool = ctx.enter_context(tc.tile_pool(name="c16", bufs=4))
        hpool = ctx.enter_context(tc.tile_pool(name="h16", bufs=3))

        A = io.tile([KSLOTS, N], F16)
        G = io.tile([KSLOTS, M], F16)
        nc.sync.dma_start(out=A, in_=a_in[:])
        nc.sync.dma_start(out=G, in_=g_in[:])

        run2 = io.tile([P, M], F16)
        nc.vector.memset(run2, -60000.0)
        Rpart = io.tile([P, NTILE], F32, name="Rpart")

        with tc.tile_pool(name="ps_main", bufs=2, space="PSUM") as psum:
            for i in range(NTILE):
                # ACT merges both PSUM halves into one fp16 tile
                C = cpool.tile([P, M], F16)
                for j in range(NCHUNK):
                    ps = psum.tile([P, MCHUNK], F32)
                    for kk in range(MCHUNK // MM_N):
                        c0 = j * MCHUNK + kk * MM_N
                        nc.tensor.matmul(
                            ps[:, kk * MM_N : (kk + 1) * MM_N],
                            A[:, i * P : (i + 1) * P],
                            G[:, c0 : c0 + MM_N],
                            start=True,
                            stop=True,
                        )
                    nc.scalar.copy(C[:, j * MCHUNK : (j + 1) * MCHUNK], ps)
                # rowmax (dist1): fold halves at 2x rate, then reduce
                H = hpool.tile([P, MCHUNK], F16)
                nc.vector.tensor_tensor(
                    H, C[:, 0:MCHUNK], C[:, MCHUNK:M], op=Alu.max
                )
                nc.vector.tensor_reduce(
                    out=Rpart[:, i : i + 1],
                    in_=H,
                    axis=X,
                    op=Alu.max,
                )
                # colmax accumulate (dist2) in fp16 2x mode
                nc.vector.tensor_tensor(run2, C, run2, op=Alu.max)

        # fold run2 across partitions on-device (gpsimd all-reduce) so only
        # [1, M] goes back over the wire
        foldt = io.tile([P, M], F16, name="foldt")
        nc.gpsimd.partition_all_reduce(foldt, run2, P, bass_isa.ReduceOp.max)

        nc.sync.dma_start(out=r1_out[:], in_=Rpart)
        nc.sync.dma_start(out=c2_out[:], in_=foldt[0:1, :])


def _split16(x):
    hi = x.astype(np.float16)
    lo = (x - hi.astype(np.float64)).astype(np.float16)
    return hi, lo


def _prep(pred, gt):
    """Build the [13, 4096] fp16 stationary/moving operand matrices."""
    p = pred.astype(np.float64)
    g = gt.astype(np.float64)
    ph, pl = _split16(p)  # [N,3] each
    gh, gl = _split16(g)
    pt = ph.astype(np.float64) + pl.astype(np.float64)
    gt_ = gh.astype(np.float64) + gl.astype(np.float64)
    pn = (pt * pt).sum(-1)  # [N]
    gn = (gt_ * gt_).sum(-1)  # [M]
    pnh, pnl = _split16(-pn)
    gnh, gnl = _split16(-gn)

    A = np.zeros((KSLOTS, N), np.float16)
    G = np.zeros((KSLOTS, M), np.float16)
    for c in range(3):
        r = 3 * c
        # (ph+pl)*(gh+gl) ~= ph*gh + ph*gl + pl*gh  (pl*gl ~ 2^-22, dropped)
        A[r + 0] = 2.0 * ph[:, c]
        A[r + 1] = 2.0 * ph[:, c]
        A[r + 2] = 2.0 * pl[:, c]
        G[r + 0] = gh[:, c]
        G[r + 1] = gl[:, c]
        G[r + 2] = gh[:, c]
    A[9] = pnh
    A[10] = pnl
    G[9] = 1.0
    G[10] = 1.0
    A[11] = 1.0
    A[12] = 1.0
    G[11] = gnh
    G[12] = gnl
    return A, G


def _get_nc():
    if "nc" not in _CACHE:
        _CACHE["nc"] = _build_nc()
    return _CACHE["nc"]


def kernel(pred_pc, gt_pc):
    global LAST_RESULT
    pred_pc = np.asarray(pred_pc)
    gt_pc = np.asarray(gt_pc)
    nc = _get_nc()
    in_maps = []
    for b in range(B):
        A, G = _prep(pred_pc[b], gt_pc[b])
        in_maps.append({"A": A, "G": G})
    res = run_bass_kernel_spmd(nc, in_maps, list(range(B)))
    LAST_RESULT = res
    losses = []
    for b in range(B):
        r1 = np.asarray(res.results[b]["R1OUT"], np.float32).reshape(P, NTILE)
        c2 = np.asarray(res.results[b]["C2OUT"], np.float32).reshape(M)  # [4096]
        d1 = np.sqrt(np.maximum(-r1, 0.0)).reshape(-1)  # [4096]
        d2 = np.sqrt(np.maximum(-c2, 0.0))  # [4096]
        loss = 0.0
        for d in (d1, d2):
            topk = np.partition(d, d.size - K1)[d.size - K1 :]
            loss += d.mean() + WEIGHT * topk.mean()
        losses.append(loss)
    return np.array(np.mean(losses), dtype=np.float32)


# revision 32
# speedup vs baseline: 1.3226x; 1.3226x over previous
"""Trainium2 Bass kernel: batched chamfer-style metric (nn_Metric_56985626083917).

Reference computation per batch b (B=8, N=M=4096, D=3):
    sqd[n,m] = |pred_n - gt_m|^2   (clamped >= 0)
    dist1 = sqrt(min_m sqd)  [N] ; dist2 = sqrt(min_n sqd)  [M]
    loss_b = mean(dist1)+mean(dist2) + 3*(mean(top2048(dist1))+mean(top2048(dist2)))
    out = mean_b loss_b

Strategy: data-parallel, one batch per NeuronCore (8 cores).
Per core the device computes zt[n,m] = -sqd[n,m] via K=13 fp16 matmuls with
error-compensated hi/lo splits (fp32-grade accuracy at full fp16 PE rate):
    zt = sum_c 2*p_c*g_c - |p|^2 - |g|^2
slots: per coord c 3 products (ph*gh, ph*gl, pl*gh; pl*gl ~ 2^-22 dropped),
plus 2 slots for -|p|^2 (hi/lo vs ones) and 2 slots for -|g|^2 -> K=13.

Main loop: statically unrolled over 32 pred tiles x 2 PSUM halves of 2048
gt columns (double-buffered). No hardware loop: tc.For_i executes at
~5-8us per instruction on this stack (measured; likely SW-decode), so
full unrolling is ~7x faster despite the larger NEFF. Benchmark reps are
additional unrolled copies of the body (V0 scheme):
    PE  : 4 matmuls per half -> PSUM [128, 2048] fp32 (zt half-tile)
    ACT : copy PSUM halves -> merged SBUF fp16 tile C [128, 4096]
    DVE : rowmax (dist1) as a TREE — 2x-mode fp16 tensor_tensor max of
          the two halves (1.1us) then a 2048-wide tensor_reduce (2.1us),
          instead of a 4096-wide 1x reduce (4.4us); plus one 2x-mode
          fp16 tensor_tensor max into run2 (dist2)
(A DVE/Pool hybrid that moved half the colmax folds to gpsimd
partition_all_reduce measured ~25% SLOWER despite a favorable per-op
microbenchmark — interleaved gpsimd reads contend for SBUF ports.)

Device outputs raw row maxes [128, 32] fp32 and the column maxes folded
on-device to [1, 4096] fp16 — keeping transfers tiny; the host (O(N)
work) does relu/sqrt, means, and exact top-k via np.partition, then
averages the 8 losses.
"""

import os
import sys

import numpy as np

for _p in ("/opt/trn_rl_repo",):
    if os.path.isdir(_p) and _p not in sys.path:
        sys.path.insert(0, _p)

import concourse.bass as bass  # noqa: E402
import concourse.bass_isa as bass_isa  # noqa: E402
import concourse.mybir as mybir  # noqa: E402
import concourse.tile as tile  # noqa: E402
from concourse import bacc  # noqa: E402
from concourse.bass_utils import run_bass_kernel_spmd  # noqa: E402

B = 8
N = 4096  # pred points per batch
M = 4096  # gt points per batch
P = 128  # partitions
KSLOTS = 13
NTILE = N // P  # 32
MCHUNK = 2048  # gt columns per PSUM half
NCHUNK = M // MCHUNK  # 2
MM_N = 512  # moving free dim per matmul (<= 1 PSUM bank)
K1 = N // 2  # top-k count (PERCENT=0.5)
WEIGHT = 3.0

F16 = mybir.dt.float16
F32 = mybir.dt.float32
Alu = mybir.AluOpType
X = mybir.AxisListType.X

LAST_RESULT = None
_CACHE = {}


def _build_nc(reps=1):
    nc = bacc.Bacc(
        "TRN2", target_bir_lowering=False, debug=False, num_devices=B
    )
    a_in = nc.dram_tensor("A", [KSLOTS, N], F16, kind="ExternalInput")
    g_in = nc.dram_tensor("G", [KSLOTS, M], F16, kind="ExternalInput")
    r1_out = nc.dram_tensor("R1OUT", [P, NTILE], F32, kind="ExternalOutput")
    c2_out = nc.dram_tensor("C2OUT", [1, M], F16, kind="ExternalOutput")

    with tile.TileContext(nc) as tc:
        for _ in range(reps):
            _body(nc, tc, a_in, g_in, r1_out, c2_out)
    nc.compile()
    return nc


def _body(nc, tc, a_in, g_in, r1_out, c2_out):
    from contextlib import ExitStack

    with ExitStack() as ctx:
        io = ctx.enter_context(tc.tile_pool(name="io", bufs=1))
        cpool = ctx.enter_context(tc.tile_pool(name="c16", bufs=4))
        hpool = ctx.enter_context(tc.tile_pool(name="h16", bufs=3))

        A = io.tile([KSLOTS, N], F16)
        G = io.tile([KSLOTS, M], F16)
        nc.sync.dma_start(out=A, in_=a_in[:])
        nc.sync.dma_start(out=G, in_=g_in[:])

        run2 = io.tile([P, M], F16)
        Rpart = io.tile([P, NTILE], F32, name="Rpart")

        with tc.tile_pool(name="ps_main", bufs=2, space="PSUM") as psum:
            for i in range(NTILE):
                # ACT merges both PSUM halves into one fp16 tile
                C = cpool.tile([P, M], F16)
                for j in range(NCHUNK):
                    ps = psum.tile([P, MCHUNK], F32)
                    for kk in range(MCHUNK // MM_N):
                        c0 = j * MCHUNK + kk * MM_N
                        nc.tensor.matmul(
                            ps[:, kk * MM_N : (kk + 1) * MM_N],
                            A[:, i * P : (i + 1) * P],
                            G[:, c0 : c0 + MM_N],
                            start=True,
                            stop=True,
                        )
                    nc.scalar.copy(C[:, j * MCHUNK : (j + 1) * MCHUNK], ps)
                # rowmax (dist1): two tree levels at 2x rate, then reduce
                H = hpool.tile([P, MCHUNK], F16)
                nc.vector.tensor_tensor(
                    H, C[:, 0:MCHUNK], C[:, MCHUNK:M], op=Alu.max
                )
                Q = MCHUNK // 2
                nc.vector.tensor_tensor(
                    H[:, 0:Q], H[:, 0:Q], H[:, Q:MCHUNK], op=Alu.max
                )
                nc.vector.tensor_reduce(
                    out=Rpart[:, i : i + 1],
                    in_=H[:, 0:Q],
                    axis=X,
                    op=Alu.max,
                )
                # colmax accumulate (dist2) in fp16 2x mode; tile 0 seeds
                # run2 with a 4x-mode copy instead of a memset + max
                if i == 0:
                    nc.vector.tensor_copy(run2, C)
                else:
                    nc.vector.tensor_tensor(run2, C, run2, op=Alu.max)

        # fold run2 across partitions on-device (gpsimd all-reduce) so only
        # [1, M] goes back over the wire
        foldt = io.tile([P, M], F16, name="foldt")
        nc.gpsimd.partition_all_reduce(foldt, run2, P, bass_isa.ReduceOp.max)

        nc.sync.dma_start(out=r1_out[:], in_=Rpart)
        nc.sync.dma_start(out=c2_out[:], in_=foldt[0:1, :])


def _split16(x):
    hi = x.astype(np.float16)
    lo = (x - hi.astype(np.float64)).astype(np.float16)
    return hi, lo


def _prep(pred, gt):
    """Build the [13, 4096] fp16 stationary/moving operand matrices."""
    p = pred.astype(np.float64)
    g = gt.astype(np.float64)
    ph, pl = _split16(p)  # [N,3] each
    gh, gl = _split16(g)
    pt = ph.astype(np.float64) + pl.astype(np.float64)
    gt_ = gh.astype(np.float64) + gl.astype(np.float64)
    pn = (pt * pt).sum(-1)  # [N]
    gn = (gt_ * gt_).sum(-1)  # [M]
    pnh, pnl = _split16(-pn)
    gnh, gnl = _split16(-gn)

    A = np.zeros((KSLOTS, N), np.float16)
    G = np.zeros((KSLOTS, M), np.float16)
    for c in range(3):
        r = 3 * c
        # (ph+pl)*(gh+gl) ~= ph*gh + ph*gl + pl*gh  (pl*gl ~ 2^-22, dropped)
        A[r + 0] = 2.0 * ph[:, c]
        A[r + 1] = 2.0 * ph[:, c]
        A[r + 2] = 2.0 * pl[:, c]
        G[r + 0] = gh[:, c]
        G[r + 1] = gl[:, c]
        G[r + 2] = gh[:, c]
    A[9] = pnh
    A[10] = pnl
    G[9] = 1.0
    G[10] = 1.0
    A[11] = 1.0
    A[12] = 1.0
    G[11] = gnh
    G[12] = gnl
    return A, G


def _get_nc():
    if "nc" not in _CACHE:
        _CACHE["nc"] = _build_nc()
    return _CACHE["nc"]


def kernel(pred_pc, gt_pc):
    global LAST_RESULT
    pred_pc = np.asarray(pred_pc)
    gt_pc = np.asarray(gt_pc)
    nc = _get_nc()
    in_maps = []
    for b in range(B):
        A, G = _prep(pred_pc[b], gt_pc[b])
        in_maps.append({"A": A, "G": G})
    res = run_bass_kernel_spmd(nc, in_maps, list(range(B)))
    LAST_RESULT = res
    losses = []
    for b in range(B):
        r1 = np.asarray(res.results[b]["R1OUT"], np.float32).reshape(P, NTILE)
        c2 = np.asarray(res.results[b]["C2OUT"], np.float32).reshape(M)  # [4096]
        d1 = np.sqrt(np.maximum(-r1, 0.0)).reshape(-1)  # [4096]
        d2 = np.sqrt(np.maximum(-c2, 0.0))  # [4096]
        loss = 0.0
        for d in (d1, d2):
            topk = np.partition(d, d.size - K1)[d.size - K1 :]
            loss += d.mean() + WEIGHT * topk.mean()
        losses.append(loss)
    return np.array(np.mean(losses), dtype=np.float32)
